# revision 1
# baseline (speedup 1.0000x reference)
"""CPPN MLP (12 -> 32 -> 32 -> 32 -> 3, per-node activations) on 8 TRN2 cores.

Data-parallel over the pixel axis. Each core processes P_CORE pixels laid out
feature-major as 4 pixel-groups on SBUF partitions:
  rhs partition (12*g + i) holds feature i of pixel-group g  (layer-1 input)
  hidden state partition layout per layer: 4 groups x 32 nodes, nodes sorted
  [sin | gauss | tanh-class] across groups so activation passes are prefix
  ranges starting at partition 0 (ISA requires start partition in {0,32,64,96}).

Matmuls use permuted block-diagonal stationary matrices (float32r = full-rate
fp32).  Per-node activation selection is done with per-partition scale/bias
operand columns on the ScalarE activation instruction plus host-side algebraic
folds into the next layer's weights:
  sigmoid(z) = 0.5*tanh(z/2) + 0.5          (stored tanh(z/2); affine folded)
  identity(z) = tanh(eps*z)/eps             (stored tanh(eps*z); 1/eps folded)
  gauss(z) = exp(-z^2/2) = (1-t)/(1+t),  t = tanh(z^2/4)   (Square+Tanh+DVE)
  sin(z): k = round(z/2pi) via fp32 magic-constant rounding (two Identity
  passes), Cody-Waite 3-term reduction on DVE, then the Sin table (+-pi domain).
All five per-node functions resolve to {Tanh, Square, Identity, Sin} which
co-reside in one activation table set (no table switching).
"""

import os
import sys

import numpy as np

_REPO = "/root/.axon_site/_ro/trn_rl_repo"
if _REPO not in sys.path and not os.path.isdir("/opt/trn_rl_repo"):
    sys.path.insert(0, _REPO)

import concourse.bacc as bacc
import concourse.bass as bass  # noqa: F401
import concourse.tile as tile
from concourse import mybir
from concourse.bass_utils import run_bass_kernel_spmd

# Pin the activation-function table to the single set containing every
# function this kernel uses ({Tanh, Square, Identity, Sin}).  Without this,
# bacc's greedy per-instruction set selection alternates between sets (Sin
# lives only in the trig/silu sets) and emits an ACT_TABLE_LOAD (~1.3us)
# per chunk.
_orig_get_tables = bacc.get_activation_tables


def _pinned_tables(arch):
    t = _orig_get_tables(arch)
    if "silu_and_others" in t:
        # act_func_set_id is the POSITION in act_info.json's set list, so
        # keep every entry (order intact) and just empty the others.
        return {name: (funcs if name == "silu_and_others" else set())
                for name, funcs in t.items()}
    return t


bacc.get_activation_tables = _pinned_tables

F32 = mybir.dt.float32
F32R = mybir.dt.float32r

P_TOTAL = 1024 * 1024
N_IN, H, N_OUT = 12, 32, 3
N_CORES = 8
P_CORE = P_TOTAL // N_CORES  # 131072
G = 4                        # pixel groups packed on partitions
PG = P_CORE // G             # 32768 pixels per group per core
CHUNK = 1024                 # pixels per group per chunk (2 PSUM banks)
MM_N = 512                   # matmul moving free dim (one PSUM bank)
MAGIC = np.float32(1.5 * 2 ** 23)   # fp32 round-to-nearest-int magic constant
INV_2PI = np.float32(1.0 / (2.0 * np.pi))
ID_EPS = np.float32(2.0 ** -18)     # identity-via-tanh input scale

# Cody-Waite split of 2*pi into 3 fp32 terms (computed in fp64)
_2PI = 2.0 * np.pi
CW1 = np.float32(_2PI)
CW2 = np.float32(_2PI - float(CW1))
CW3 = np.float32(_2PI - float(CW1) - float(CW2))

# class codes: 0 = sin, 1 = gauss, 2 = tanh-class (tanh/sigmoid/identity)
def _cls_of_act(a):
    return {3: 0, 4: 1}.get(int(a), 2)


def _sorted_layout(act):
    """Order the H nodes by [sin | gauss | rest]; return (perm, n_sin, n_gauss).
    perm[j] = original node index placed at sorted slot j."""
    cls = np.array([_cls_of_act(a) for a in act])
    perm = np.argsort(cls, kind="stable")
    return perm, int((cls == 0).sum()), int((cls == 1).sum())


class _Plan:
    """Host-side folded weights + per-layer layouts. All float64 math."""

    def __init__(self, bias_in, W1, b1, act1, W2, b2, act2, W3, b3, act3,
                 Wout, bout):
        layers = [(W1, b1, act1), (W2, b2, act2), (W3, b3, act3)]
        self.perms, self.nsin, self.ngauss = [], [], []
        self.lhsT = []          # device stationary matrices (np.float32)
        self.cols = []          # per-layer dict of [128] operand columns
        # incoming per-node output transform: h_true = alpha*stored + beta
        in_alpha = np.ones(N_IN, dtype=np.float64)
        in_beta = np.asarray(bias_in, dtype=np.float64)  # h0 = x + bias_in
        in_dim = N_IN
        in_layout = None  # for L1 the input layout is the fixed feature order

        for li, (W, b, act) in enumerate(layers):
            W = np.asarray(W, dtype=np.float64)
            b = np.asarray(b, dtype=np.float64)
            act = np.asarray(act)
            perm, ns, ng = _sorted_layout(act)
            self.perms.append(perm)
            self.nsin.append(ns)
            self.ngauss.append(ng)

            # effective weights / bias absorbing incoming transforms
            W_eff = W * in_alpha[:, None]                  # [in_dim, H]
            b_eff = b + in_beta @ W                        # [H]

            # device stationary: block diagonal over groups with node sort
            K = G * in_dim
            lt = np.zeros((K, 128), dtype=np.float64)
            for g in range(G):
                for j in range(H):
                    node = perm[j]
                    m = self._row(li, g, j)
                    if li == 0:
                        rows = np.arange(in_dim) + in_dim * g
                        lt[rows, m] = W_eff[:, node]
                    else:
                        for k_in in range(in_dim):
                            kpart = in_layout[g][k_in]
                            lt[kpart, m] = W_eff[k_in, node]
            self.lhsT.append(lt.astype(np.float32))

            # activation operand columns, indexed by device partition
            tanh_scale = np.zeros(128, dtype=np.float64)
            tanh_bias = np.zeros(128, dtype=np.float64)
            sq_scale = np.zeros(128, dtype=np.float64)
            sq_bias = np.zeros(128, dtype=np.float64)
            p1_bias = np.full(128, float(MAGIC), dtype=np.float64)
            sin_bias = np.zeros(128, dtype=np.float64)
            out_alpha = np.ones(H, dtype=np.float64)
            out_beta = np.zeros(H, dtype=np.float64)
            for j in range(H):
                node = perm[j]
                a = int(act[node])
                be = b_eff[node]
                for g in range(G):
                    m = self._row(li, g, j)
                    if a == 1:        # tanh
                        tanh_scale[m] = 1.0
                        tanh_bias[m] = be
                    elif a == 2:      # sigmoid -> tanh(z/2)
                        tanh_scale[m] = 0.5
                        tanh_bias[m] = 0.5 * be
                    elif a == 0:      # identity -> tanh(eps*z)
                        tanh_scale[m] = float(ID_EPS)
                        tanh_bias[m] = float(ID_EPS) * be
                    elif a == 3:      # sin
                        sin_bias[m] = be
                    elif a == 4:      # gauss: y=(z/2)^2 then tanh
                        sq_scale[m] = 0.5
                        sq_bias[m] = 0.5 * be
                if a == 1:
                    out_alpha[node], out_beta[node] = 1.0, 0.0
                elif a == 2:
                    out_alpha[node], out_beta[node] = 0.5, 0.5
                elif a == 0:
                    out_alpha[node], out_beta[node] = 1.0 / float(ID_EPS), 0.0
                elif a == 3:
                    out_alpha[node], out_beta[node] = 1.0, 0.0
                elif a == 4:
                    out_alpha[node], out_beta[node] = 1.0, 0.0
            self.cols.append({
                "tanh_scale": tanh_scale, "tanh_bias": tanh_bias,
                "sq_scale": sq_scale, "sq_bias": sq_bias,
                "p1_bias": p1_bias, "sin_bias": sin_bias,
            })

            # next layer's incoming transform, in SORTED node order per device
            # partition -> but folds are per node; store per-node arrays and
            # the partition layout for the next lhsT build.
            in_alpha = out_alpha
            in_beta = out_beta
            in_dim = H
            # partition index of (g, sorted-slot j) for this layer's output
            in_layout = [[self._row(li, g, j) for j in range(H)]
                         for g in range(G)]
            # reorder alpha/beta to sorted-slot order for the next W_eff
            in_alpha = out_alpha[perm]
            in_beta = out_beta[perm]
            # next layer's W rows must be permuted accordingly
            if li < 2:
                layers[li + 1] = (np.asarray(layers[li + 1][0])[perm, :],
                                  layers[li + 1][1], layers[li + 1][2])
            else:
                self._wout_perm = perm

        # output layer
        Wo = np.asarray(Wout, dtype=np.float64)[self._wout_perm, :]
        bo = np.asarray(bout, dtype=np.float64)
        Wo_eff = Wo * in_alpha[:, None]
        bo_eff = bo + in_beta @ Wo
        lt = np.zeros((128, 32), dtype=np.float64)
        for g in range(G):
            for j in range(H):
                kpart = in_layout[g][j]
                for o in range(N_OUT):
                    lt[kpart, 3 * g + o] = Wo_eff[j, o]
        self.lhsT_out = lt.astype(np.float32)
        out_bias = np.zeros(128, dtype=np.float64)
        for q in range(4):
            for g in range(G):
                for o in range(N_OUT):
                    out_bias[32 * q + 3 * g + o] = bo_eff[o]
        self.out_bias = out_bias

        # pack all operand columns into one [128, 32] block
        colblk = np.zeros((128, 32), dtype=np.float64)
        for li in range(3):
            c = self.cols[li]
            colblk[:, 8 * li + 0] = c["tanh_scale"]
            colblk[:, 8 * li + 1] = c["tanh_bias"]
            colblk[:, 8 * li + 2] = c["sq_scale"]
            colblk[:, 8 * li + 3] = c["sq_bias"]
            colblk[:, 8 * li + 4] = c["p1_bias"]
            colblk[:, 8 * li + 5] = c["sin_bias"]
        colblk[:, 24] = self.out_bias
        colblk[:, 25] = -float(MAGIC)
        colblk[:, 26] = float(INV_2PI)
        colblk[:, 27] = float(MAGIC)
        self.colblk = colblk.astype(np.float32)

    @staticmethod
    def _row(li, g, j):
        """Device partition of sorted-slot j, group g (layer output layout).
        Rows are class-sorted ACROSS groups: slot j occupies partitions
        4*j + g."""
        return 4 * j + g

    def prefix_sizes(self, li):
        ns, ng = self.nsin[li], self.ngauss[li]
        return 4 * ns, 4 * (ns + ng)


def _build_program(nsin, ngauss, p_core=P_CORE, chunk=CHUNK,
                   use_fp32r=False):
    """Build the bass module. Program structure depends only on the per-layer
    (n_sin, n_gauss) counts (prefix range lengths), not on weight values."""
    pg = p_core // G
    nchunk = pg // chunk
    nhalf = chunk // MM_N
    assert chunk % MM_N == 0 and pg % chunk == 0

    nc = bacc.Bacc("TRN2", target_bir_lowering=False, debug=False,
                   num_devices=N_CORES)
    xT = nc.dram_tensor("xT", [G * N_IN, pg], F32, kind="ExternalInput").ap()
    cst = nc.dram_tensor("cst", [128, 480], F32, kind="ExternalInput").ap()
    yT = nc.dram_tensor("yT", [12, pg], F32, kind="ExternalOutput").ap()

    with tile.TileContext(nc) as tc:
        cpool = tc.alloc_tile_pool(name="consts", bufs=1)
        wdt = F32R if use_fp32r else F32
        wst_t = cpool.tile([128, 416], wdt, tag="wst")
        cc_t = cpool.tile([128, 64], F32, tag="cc")
        if use_fp32r:
            nc.gpsimd.dma_start(out=wst_t[:], in_=cst[:, 0:416])
        else:
            nc.sync.dma_start(out=wst_t[:], in_=cst[:, 0:416])
        nc.sync.dma_start(out=cc_t[:], in_=cst[:, 416:480])
        w1_t = wst_t[:, 0:128]
        w2_t = wst_t[:, 128:256]
        w3_t = wst_t[:, 256:384]
        wo_t = wst_t[:, 384:416]
        col_t = cc_t[:, 0:32]

        xpool = tc.alloc_tile_pool(name="xin", bufs=4)
        hpool = tc.alloc_tile_pool(name="h", bufs=8)
        spool = tc.alloc_tile_pool(name="scratch", bufs=3)
        opool = tc.alloc_tile_pool(name="osb", bufs=2)
        ppool = tc.alloc_tile_pool(name="psum", bufs=3, space="PSUM")
        oppool = tc.alloc_tile_pool(name="psum_o", bufs=2, space="PSUM")

        w_tiles = [w1_t, w2_t, w3_t]
        osb = None
        h_live = {}     # (chunk, li) -> produced tile (li 0 == x input)
        pso_live = {}   # chunk-pair -> psum_o tile

        def emit_load(c):
            x_t = xpool.tile([G * N_IN, chunk], F32R if use_fp32r else F32,
                             tag="x")
            if use_fp32r:
                nc.gpsimd.dma_start(
                    out=x_t[:], in_=xT[:, c * chunk:(c + 1) * chunk])
            else:
                nc.sync.dma_start(
                    out=x_t[:], in_=xT[:, c * chunk:(c + 1) * chunk])
            h_live[(c, 0)] = x_t

        def emit_layer(c, li):
            h_prev = h_live.pop((c, li))
            pref_s, pref_sg = 4 * nsin[li], 4 * (nsin[li] + ngauss[li])
            kdim = G * N_IN if li == 0 else 128
            ps = ppool.tile([128, chunk], F32, tag="pre")
            wt = w_tiles[li]
            for hh in range(nhalf):
                sl = slice(hh * MM_N, (hh + 1) * MM_N)
                nc.tensor.matmul(
                    ps[:, sl],
                    wt[0:kdim, :],
                    h_prev[0:kdim, sl],
                    start=True, stop=True,
                )
            h = hpool.tile([128, chunk], F32R if use_fp32r else F32,
                           tag="h")
            cb = 8 * li
            # 1) tanh-class over all 128 rows (junk on sin/gauss rows)
            nc.scalar.activation(
                h[:], ps[:], mybir.ActivationFunctionType.Tanh,
                bias=col_t[:, cb + 1:cb + 2],
                scale=col_t[:, cb + 0:cb + 1],
            )
            if pref_sg > pref_s:
                # 2) gauss: y = ((z)/2)^2 ; t = tanh(y);
                #    h = (1-t)/(1+t) = exp(-z^2/2)
                y_t = spool.tile([128, chunk], F32, tag="sq")
                nc.scalar.activation(
                    y_t[0:pref_sg, :], ps[0:pref_sg, :],
                    mybir.ActivationFunctionType.Square,
                    bias=col_t[0:pref_sg, cb + 3:cb + 4],
                    scale=col_t[0:pref_sg, cb + 2:cb + 3],
                )
                t_t = spool.tile([128, chunk], F32, tag="tg")
                nc.scalar.activation(
                    t_t[0:pref_sg, :], y_t[0:pref_sg, :],
                    mybir.ActivationFunctionType.Tanh,
                )
                num_t = spool.tile([128, chunk], F32, tag="num")
                den_t = spool.tile([128, chunk], F32, tag="den")
                nc.vector.tensor_scalar(
                    num_t[0:pref_sg, :], t_t[0:pref_sg, :],
                    -1.0, 1.0, mybir.AluOpType.mult, mybir.AluOpType.add)
                nc.vector.tensor_scalar(
                    den_t[0:pref_sg, :], t_t[0:pref_sg, :],
                    1.0, 1.0, mybir.AluOpType.mult, mybir.AluOpType.add)
                # den is in [1, 2]: approx-fast's ~51-ULP reciprocal gives
                # ~3e-6 relative error on the gauss branch - well inside
                # tolerance, and one DVE op instead of three.
                rin_t = spool.tile([128, chunk], F32, tag="rin")
                nc.vector.reciprocal_approx_fast(
                    rin_t[0:pref_sg, :], den_t[0:pref_sg, :])
                nc.vector.tensor_tensor(
                    h[0:pref_sg, :], num_t[0:pref_sg, :],
                    rin_t[0:pref_sg, :], mybir.AluOpType.mult)
            if pref_s > 0:
                # 3) sin with range reduction to [-pi, pi]:
                #    t0 = z = u + b;  k = round(z/2pi) via magic const;
                #    ur = z - 2pi*k (Cody-Waite);  h = Sin(ur)
                t0 = spool.tile([128, chunk], F32, tag="t0")
                nc.vector.tensor_scalar(
                    t0[0:pref_s, :], ps[0:pref_s, :],
                    col_t[0:pref_s, cb + 5:cb + 6], None,
                    mybir.AluOpType.add)
                t1 = spool.tile([128, chunk], F32, tag="t1")
                nc.scalar.activation(
                    t1[0:pref_s, :], t0[0:pref_s, :],
                    mybir.ActivationFunctionType.Identity,
                    bias=col_t[0:pref_s, 27:28],
                    scale=col_t[0:pref_s, 26:27],
                )
                kr = spool.tile([128, chunk], F32, tag="kr")
                nc.scalar.activation(
                    kr[0:pref_s, :], t1[0:pref_s, :],
                    mybir.ActivationFunctionType.Identity,
                    bias=col_t[0:pref_s, 25:26],
                )
                ur = spool.tile([128, chunk], F32, tag="ur")
                nc.vector.cody_waite_cascade(
                    ur[0:pref_s, :], t0[0:pref_s, :], kr[0:pref_s, :],
                    float(CW1), float(CW2), float(CW3))
                nc.scalar.activation(
                    h[0:pref_s, :], ur[0:pref_s, :],
                    mybir.ActivationFunctionType.Sin,
                )
            h_live[(c, li + 1)] = h

        def emit_out(c):
            # output layer: quadrant-packed [12,512] matmuls
            h_prev = h_live.pop((c, 3))
            q0 = 2 * (c % 2)
            if q0 == 0:
                pso_live[c // 2] = oppool.tile([128, MM_N], F32, tag="preo",
                                               name="pso")
            pso = pso_live[c // 2]
            for hh in range(nhalf):
                q = q0 + hh
                # fp32r forbids nonzero column tile_position: run the small
                # output-layer matmuls in plain fp32 (bitcast is free).
                nc.tensor.matmul(
                    pso[32 * q:32 * q + 32, :],
                    wo_t.bitcast(F32) if use_fp32r else wo_t,
                    h_prev[:, hh * MM_N:(hh + 1) * MM_N].bitcast(F32)
                    if use_fp32r
                    else h_prev[:, hh * MM_N:(hh + 1) * MM_N],
                    start=True, stop=True,
                    tile_position=(0, 32 * q),
                )
            if q0 == 2:
                pso_live.pop(c // 2)
                osb = opool.tile([128, MM_N], F32, tag="osb")
                nc.scalar.activation(
                    osb[:], pso[:],
                    mybir.ActivationFunctionType.Tanh,
                    bias=col_t[:, 24:25],
                )
                base = (c - 1) * chunk
                for q in range(4):
                    nc.sync.dma_start(
                        out=yT[:, base + q * MM_N: base + (q + 1) * MM_N],
                        in_=osb[32 * q:32 * q + 12, :])

        # Software-pipelined emission: at step t the units
        # (t, load), (t, L1), (t-1, L2), (t-2, L3), (t-3, out) are emitted,
        # so each engine's program order interleaves 4 chunks and the
        # scheduler (whose priorities follow emission order) can keep every
        # engine busy despite the deep per-unit dependency chains.
        for t in range(nchunk + 3):
            if t < nchunk:
                emit_load(t)
                emit_layer(t, 0)
            if 1 <= t and t - 1 < nchunk:
                emit_layer(t - 1, 1)
            if 2 <= t and t - 2 < nchunk:
                emit_layer(t - 2, 2)
            if 3 <= t and t - 3 < nchunk:
                emit_out(t - 3)

        for p in (oppool, ppool, opool, spool, hpool, xpool, cpool):
            p.release()

    nc.compile()
    return nc


_PROGRAM_CACHE = {}


def _get_program(nsin, ngauss, p_core=P_CORE, chunk=CHUNK, use_fp32r=False):
    key = (tuple(nsin), tuple(ngauss), p_core, chunk, use_fp32r)
    if key not in _PROGRAM_CACHE:
        _PROGRAM_CACHE[key] = _build_program(nsin, ngauss, p_core, chunk,
                                             use_fp32r=use_fp32r)
    return _PROGRAM_CACHE[key]


def make_in_maps(inputs, plan, p_core=P_CORE, n_cores=N_CORES):
    """Shard + transpose the pixel data; replicate constants."""
    x = np.ascontiguousarray(np.asarray(inputs["inputs"], dtype=np.float32))
    pg = p_core // G
    in_maps = []
    for core in range(n_cores):
        xc = x[core * p_core:(core + 1) * p_core]          # [p_core, 12]
        xg = xc.reshape(G, pg, N_IN)                        # [G, pg, 12]
        xT = np.ascontiguousarray(
            xg.transpose(0, 2, 1).reshape(G * N_IN, pg))    # [48, pg]
        cst = np.zeros((128, 480), dtype=np.float32)
        cst[0:G * N_IN, 0:128] = plan.lhsT[0]
        cst[:, 128:256] = plan.lhsT[1]
        cst[:, 256:384] = plan.lhsT[2]
        cst[:, 384:416] = plan.lhsT_out
        cst[:, 416:448] = plan.colblk
        in_maps.append({"xT": xT, "cst": cst})
    return in_maps


def assemble_output(results, p_core=P_CORE, n_cores=N_CORES):
    pg = p_core // G
    out = np.empty((p_core * n_cores, N_OUT), dtype=np.float32)
    for core in range(n_cores):
        yT = results[core]["yT"]                            # [12, pg]
        yc = yT.reshape(G, N_OUT, pg).transpose(0, 2, 1)    # [G, pg, 3]
        out[core * p_core:(core + 1) * p_core] = yc.reshape(p_core, N_OUT)
    return out


def make_plan(inputs):
    return _Plan(
        inputs["bias_in"], inputs["W1"], inputs["b1"], inputs["act1"],
        inputs["W2"], inputs["b2"], inputs["act2"],
        inputs["W3"], inputs["b3"], inputs["act3"],
        inputs["Wout"], inputs["bout"])


def run(inputs, trace=False, use_fp32r=False, **spmd_kwargs):
    plan = make_plan(inputs)
    nc = _get_program(plan.nsin, plan.ngauss, use_fp32r=use_fp32r)
    in_maps = make_in_maps(inputs, plan)
    res = run_bass_kernel_spmd(nc, in_maps, list(range(N_CORES)),
                               trace=trace, **spmd_kwargs)
    return assemble_output(res.results), res


def kernel(**inputs) -> np.ndarray:
    out, _ = run(inputs, trace=False)
    return out



# revision 11
# speedup vs baseline: 1.4138x; 1.4138x over previous
"""CPPN MLP (12 -> 32 -> 32 -> 32 -> 3, per-node activations) on 8 TRN2 cores.

Data-parallel over the pixel axis; feature-major layout with G=4 pixel groups
on SBUF partitions (rhs partition 12*g+i holds feature i of group g for L1).

Per layer the 128 hidden rows (4 groups x 32 nodes, slot j -> partition
4*j+g) are class-sorted [gauss | tanh-class | sin].  Per chunk-layer the
ScalarE runs ONE dense main pass:
  Tanh over [0 : 128-4*nsin]  (per-row scale/bias operand columns: tanh
  (1,b), sigmoid (.5,.5b) [fold 0.5,0.5], identity (eps, eps*b) [fold 1/eps];
  junk on the gauss rows)
The sparse gauss/sin rows are DMA-stacked across k consecutive chunks into
dense SBUF stack tiles (k = 2 or 4 chosen per class so k*rows <= 128), where
per k chunks ONE dense pass each runs:
  sin:   ADD_RANGE_WRAP (DVE, custom ops cannot read PSUM - hence the DMA
         staging) then Sin(w + b) [bias column, stacked pattern]
  gauss: Square in-place (y=((z+b)/2)^2), Tanh, den=t+1 (DVE TS),
         r=recip_approx_fast(den) -> f32r, with exp(-u)=2/(1+tanh(u/2))-1
         folded (2,-1) into the next layer's weights
then k DMAs scatter the results back to each chunk's f32r h tile.

Matmuls are fp32r (full-rate fp32, ~1e-4) for the three hidden layers; the
output layer is bf16 (h3 copied to bf16 on DVE) quadrant-packed.
"""

import os
import sys

import numpy as np

_REPO = "/root/.axon_site/_ro/trn_rl_repo"
if _REPO not in sys.path and not os.path.isdir("/opt/trn_rl_repo"):
    sys.path.insert(0, _REPO)

import concourse.bacc as bacc
import concourse.bass as bass  # noqa: F401
import concourse.tile as tile
from concourse import mybir
from concourse.bass_utils import run_bass_kernel_spmd
from concourse.dve_ops import RECIP_APPROX_FAST_CONSTS as _RC
from concourse.dve_ops import RECIPROCAL_APPROX_FAST as _RF

# Pin the activation-function table to the single set containing every
# function this kernel uses ({Tanh, Square, Sin}).
_orig_get_tables = bacc.get_activation_tables


def _pinned_tables(arch):
    t = _orig_get_tables(arch)
    if "silu_and_others" in t:
        return {name: (funcs if name == "silu_and_others" else set())
                for name, funcs in t.items()}
    return t


bacc.get_activation_tables = _pinned_tables

F32 = mybir.dt.float32
F32R = mybir.dt.float32r
BF16 = mybir.dt.bfloat16
AF = mybir.ActivationFunctionType

P_TOTAL = 1024 * 1024
N_IN, H, N_OUT = 12, 32, 3
N_CORES = 8
P_CORE = P_TOTAL // N_CORES  # 131072
G = 4                        # pixel groups packed on partitions
PG = P_CORE // G             # 32768 pixels per group per core
CHUNK = 1024                 # pixels per group per chunk (2 PSUM banks)
MM_N = 512                   # matmul moving free dim (one PSUM bank)
GROUP = 4                    # chunks per pipeline group (stacking window)
ID_EPS = np.float32(2.0 ** -18)     # identity-via-tanh input scale
PI = float(np.pi)

# class codes determine the sort: 0 = gauss, 1 = tanh-class, 2 = sin
_CLS = {4: 0, 1: 1, 2: 1, 0: 1, 3: 2}


def _stack_k(rows, group):
    if rows == 0:
        return 0
    k = min(group, 128 // rows)
    return 4 if k >= 4 else (2 if k >= 2 else 1)


class _Plan:
    """Host-side folded weights + per-layer layouts. All float64 math."""

    def __init__(self, x, bias_in, W1, b1, act1, W2, b2, act2, W3, b3, act3,
                 Wout, bout):
        layers = [(np.asarray(W1, np.float64), np.asarray(b1, np.float64),
                   np.asarray(act1)),
                  (np.asarray(W2, np.float64), np.asarray(b2, np.float64),
                   np.asarray(act2)),
                  (np.asarray(W3, np.float64), np.asarray(b3, np.float64),
                   np.asarray(act3))]
        self.nsin, self.ngauss, self.nwrap = [], [], []
        self.perms = []
        self.lhsT = []

        # rigorous |z_true + b| bounds for wrap counts
        x64 = np.asarray(x, np.float64)
        bin64 = np.asarray(bias_in, np.float64)
        h0_max = np.abs(x64).max(axis=0) + np.abs(bin64)
        bounds = []
        maxh = h0_max
        for li, (W, b, act) in enumerate(layers):
            zb = np.abs(W).T @ maxh + np.abs(b)
            if li == 0:
                need = [n for n in range(H) if act[n] in (0, 3)]
                if need:
                    z1 = (x64 + bin64) @ W[:, need] + b[need]
                    zb[need] = np.abs(z1).max(axis=0)
            bounds.append(zb)
            mh = np.ones(H)
            for n in range(H):
                if act[n] == 0:
                    mh[n] = zb[n]
            maxh = mh

        in_alpha = np.ones(N_IN, dtype=np.float64)
        in_beta = bin64.copy()
        in_dim = N_IN
        in_layout = None
        colblk = np.zeros((128, 8), dtype=np.float64)

        for li, (W, b, act) in enumerate(layers):
            cls = np.array([_CLS[int(a)] for a in act])
            perm = np.argsort(cls, kind="stable")
            ns = int((cls == 2).sum())
            ng = int((cls == 0).sum())
            self.perms.append(perm)
            self.nsin.append(ns)
            self.ngauss.append(ng)

            W_eff = W * in_alpha[:, None]
            b_eff = b + in_beta @ W

            sin_nodes = [n for n in range(H) if act[n] == 3]
            if sin_nodes:
                m = max(bounds[li][n] + abs(b_eff[n]) for n in sin_nodes)
                self.nwrap.append(max(1, int(np.ceil((m / PI - 1.0) / 2.0))))
            else:
                self.nwrap.append(0)

            K = G * in_dim
            lt = np.zeros((K, 128), dtype=np.float64)
            for g in range(G):
                for j in range(H):
                    node = perm[j]
                    m_ = 4 * j + g
                    if li == 0:
                        rows = np.arange(in_dim) + in_dim * g
                        lt[rows, m_] = W_eff[:, node]
                    else:
                        for k_in in range(in_dim):
                            lt[in_layout[g][k_in], m_] = W_eff[k_in, node]
            self.lhsT.append(lt.astype(np.float32))

            # main-pass operand columns [tanh_scale, tanh_bias]: sin and
            # gauss rows stage eps*(z+b) through tanh (inverted on the stack)
            out_alpha = np.ones(H, dtype=np.float64)
            out_beta = np.zeros(H, dtype=np.float64)
            cb = 2 * li
            for j in range(H):
                node = perm[j]
                a = int(act[node])
                be = b_eff[node]
                for g in range(G):
                    m_ = 4 * j + g
                    if a == 1:
                        colblk[m_, cb + 0] = 1.0
                        colblk[m_, cb + 1] = be
                    elif a == 2:
                        colblk[m_, cb + 0] = 0.5
                        colblk[m_, cb + 1] = 0.5 * be
                    else:   # identity, gauss, sin: eps*(z+b)
                        colblk[m_, cb + 0] = float(ID_EPS)
                        colblk[m_, cb + 1] = float(ID_EPS) * be
                if a == 4:
                    out_alpha[node], out_beta[node] = 2.0, -1.0
                elif a == 2:
                    out_alpha[node], out_beta[node] = 0.5, 0.5
                elif a == 0:
                    out_alpha[node], out_beta[node] = 1.0 / float(ID_EPS), 0.0

            in_dim = H
            in_layout = [[4 * j + g for j in range(H)] for g in range(G)]
            in_alpha = out_alpha[perm]
            in_beta = out_beta[perm]
            if li < 2:
                layers[li + 1] = (np.asarray(layers[li + 1][0])[perm, :],
                                  layers[li + 1][1], layers[li + 1][2])
            else:
                self._wout_perm = perm

        # output layer (quadrant-packed, bf16)
        Wo = np.asarray(Wout, dtype=np.float64)[self._wout_perm, :]
        bo = np.asarray(bout, dtype=np.float64)
        Wo_eff = Wo * in_alpha[:, None]
        bo_eff = bo + in_beta @ Wo
        lt = np.zeros((128, 32), dtype=np.float64)
        for g in range(G):
            for j in range(H):
                kpart = in_layout[g][j]
                for o in range(N_OUT):
                    lt[kpart, 3 * g + o] = Wo_eff[j, o]
        self.lhsT_out = lt.astype(np.float32)
        out_bias = np.zeros(128, dtype=np.float64)
        for q in range(4):
            for g in range(G):
                for o in range(N_OUT):
                    out_bias[32 * q + 3 * g + o] = bo_eff[o]
        colblk[:, 6] = out_bias
        self.colblk = colblk.astype(np.float32)


def _build_program(nsin, ngauss, nwrap, p_core=P_CORE, chunk=CHUNK):
    """Program structure depends only on (nsin, ngauss, nwrap) per layer."""
    pg = p_core // G
    nchunk = pg // chunk
    nhalf = chunk // MM_N
    group = min(GROUP, nchunk)
    ngroups = (nchunk + group - 1) // group
    assert chunk % MM_N == 0 and pg % chunk == 0

    nc = bacc.Bacc("TRN2", target_bir_lowering=False, debug=False,
                   num_devices=N_CORES)
    xT = nc.dram_tensor("xT", [G * N_IN, pg], F32, kind="ExternalInput").ap()
    cst = nc.dram_tensor("cst", [128, 448], F32, kind="ExternalInput").ap()
    yT = nc.dram_tensor("yT", [12, pg], F32, kind="ExternalOutput").ap()

    with tile.TileContext(nc) as tc:
        cpool = tc.alloc_tile_pool(name="consts", bufs=1)
        wst_t = cpool.tile([128, 416], F32R, tag="wst")
        wob_t = cpool.tile([128, 32], BF16, tag="wob")
        cc_t = cpool.tile([128, 8], F32, tag="cc")
        nc.gpsimd.dma_start(out=wst_t[:], in_=cst[:, 0:416])
        nc.sync.dma_start(out=cc_t[:], in_=cst[:, 416:424])
        # bf16 copy of the output stationary (DVE converts)
        nc.vector.tensor_copy(wob_t[:], wst_t[:, 384:416].bitcast(F32))
        w_tiles = [wst_t[:, 0:128], wst_t[:, 128:256], wst_t[:, 256:384]]
        col_t = cc_t[:, 0:8]

        xpool = tc.alloc_tile_pool(name="xin", bufs=2 * group)
        hpool = tc.alloc_tile_pool(name="h", bufs=3 * group + 2)
        bpool = tc.alloc_tile_pool(name="hb16", bufs=group + 2)
        stkpool = tc.alloc_tile_pool(name="stk", bufs=4)
        scrpool = tc.alloc_tile_pool(name="scr", bufs=2)
        opool = tc.alloc_tile_pool(name="osb", bufs=2)
        ppool = tc.alloc_tile_pool(name="psum", bufs=3, space="PSUM")
        oppool = tc.alloc_tile_pool(name="psum_o", bufs=2, space="PSUM")

        h_live = {}
        pso_live = {}
        sin_pend = {0: [], 1: [], 2: []}    # li -> [(slot_b, h_tile)]
        sin_stk = {}
        gau_pend = {0: [], 1: [], 2: []}
        gau_stk = {}

        def emit_load(c):
            x_t = xpool.tile([G * N_IN, chunk], F32R, tag="x")
            nc.gpsimd.dma_start(out=x_t[:],
                                in_=xT[:, c * chunk:(c + 1) * chunk])
            h_live[(c, 0)] = x_t

        def flush_sin(li):
            pend = sin_pend[li]
            if not pend:
                return
            S4 = 4 * nsin[li]
            used = len(pend) * S4
            stk = sin_stk[li]
            # stack holds eps*(z+b); unscale, wrap into [-pi, pi], Sin
            wscr = scrpool.tile([128, chunk], F32, tag="wscr")
            nc.vector.tensor_scalar(
                wscr[0:used, :], stk[0:used, :], 1.0 / float(ID_EPS), None,
                mybir.AluOpType.mult)
            src, dst = wscr, stk
            for _ in range(nwrap[li]):
                nc.vector.add_range_wrap(dst[0:used, :], src[0:used, :],
                                         0.0, PI, 2 * PI)
                src, dst = dst, src
            nc.scalar.activation(
                dst[0:used, :].bitcast(F32R), src[0:used, :], AF.Sin)
            for b, h in pend:
                nc.sync.dma_start(
                    out=h[128 - S4:128, :],
                    in_=dst[b * S4:(b + 1) * S4, :].bitcast(F32R))
            pend.clear()

        def flush_gauss(li):
            pend = gau_pend[li]
            if not pend:
                return
            G4 = 4 * ngauss[li]
            used = len(pend) * G4
            stk = gau_stk[li]
            # stack holds eps*(z+b): y = ((z+b)/2)^2 in place, t = tanh(y)
            nc.scalar.activation(
                stk[0:used, :], stk[0:used, :], AF.Square,
                scale=0.5 / float(ID_EPS))
            nc.scalar.activation(
                stk[0:used, :], stk[0:used, :], AF.Tanh)
            den = scrpool.tile([128, chunk], F32, tag="den")
            nc.vector.tensor_scalar(
                den[0:used, :], stk[0:used, :], 1.0, None,
                mybir.AluOpType.add)
            nc.vector._custom_dve(
                _RF, out=stk[0:used, :].bitcast(F32R), in0=den[0:used, :],
                s0=_RC["s0"], s1=_RC["s1"], imm2=_RC["imm2"])
            for b, h in pend:
                nc.sync.dma_start(
                    out=h[0:G4, :],
                    in_=stk[b * G4:(b + 1) * G4, :].bitcast(F32R))
            pend.clear()

        def emit_layer(c, li, last_in_group):
            h_prev = h_live.pop((c, li))
            ns, ng = nsin[li], ngauss[li]
            S4, G4 = 4 * ns, 4 * ng
            ks = _stack_k(S4, group)
            kg = _stack_k(G4, group)
            kdim = G * N_IN if li == 0 else 128
            cb = 2 * li

            ps = ppool.tile([128, chunk], F32, tag="pre")
            wt = w_tiles[li]
            for hh in range(nhalf):
                sl = slice(hh * MM_N, (hh + 1) * MM_N)
                nc.tensor.matmul(
                    ps[:, sl],
                    wt[0:kdim, :],
                    h_prev[0:kdim, sl],
                    start=True, stop=True,
                )
            h = hpool.tile([128, chunk], F32R, tag="h")
            # one dense main pass over all rows: tanh-class real values;
            # gauss/sin rows staged as tanh(eps*(z+b)) ~= eps*(z+b)
            nc.scalar.activation(
                h[:, :], ps[:, :], AF.Tanh,
                bias=col_t[:, cb + 1:cb + 2],
                scale=col_t[:, cb + 0:cb + 1],
            )
            # stack the sparse class rows (SBUF -> SBUF)
            if S4:
                if not sin_pend[li]:
                    sin_stk[li] = stkpool.tile([128, chunk], F32, tag="sstk",
                                               name="sstk")
                b = len(sin_pend[li])
                nc.sync.dma_start(out=sin_stk[li][b * S4:(b + 1) * S4, :],
                                  in_=h[128 - S4:128, :].bitcast(F32))
                sin_pend[li].append((b, h))
                if len(sin_pend[li]) == ks or last_in_group:
                    flush_sin(li)
            if G4:
                if not gau_pend[li]:
                    gau_stk[li] = stkpool.tile([128, chunk], F32, tag="gstk",
                                               name="gstk")
                b = len(gau_pend[li])
                nc.sync.dma_start(out=gau_stk[li][b * G4:(b + 1) * G4, :],
                                  in_=h[0:G4, :].bitcast(F32))
                gau_pend[li].append((b, h))
                if len(gau_pend[li]) == kg or last_in_group:
                    flush_gauss(li)
            h_live[(c, li + 1)] = h

        def emit_h3_bf16(c):
            h3 = h_live.pop((c, 3))
            hb = bpool.tile([128, chunk], BF16, tag="hb")
            nc.vector.tensor_copy(hb[:], h3[:].bitcast(F32))
            h_live[(c, 4)] = hb

        def emit_out(c):
            h_prev = h_live.pop((c, 4))
            q0 = 2 * (c % 2)
            if q0 == 0:
                pso_live[c // 2] = oppool.tile([128, MM_N], F32, tag="preo",
                                               name="pso")
            pso = pso_live[c // 2]
            for hh in range(nhalf):
                q = q0 + hh
                nc.tensor.matmul(
                    pso[32 * q:32 * q + 32, :],
                    wob_t,
                    h_prev[:, hh * MM_N:(hh + 1) * MM_N],
                    start=True, stop=True,
                    tile_position=(0, 32 * q),
                )
            if q0 == 2:
                pso_live.pop(c // 2)
                osb = opool.tile([128, MM_N], F32, tag="osb")
                nc.scalar.activation(
                    osb[:], pso[:], AF.Tanh,
                    bias=col_t[:, 6:7],
                )
                base = (c - 1) * chunk
                for q in range(4):
                    nc.sync.dma_start(
                        out=yT[:, base + q * MM_N: base + (q + 1) * MM_N],
                        in_=osb[32 * q:32 * q + 12, :])

        def grp(t):
            return range(t * group, min((t + 1) * group, nchunk))

        # group-granular software pipeline: step t emits loads+L1 for group
        # t, L2 for t-1, L3 for t-2, out for t-3. All stacked-class writes
        # to a group's h tiles complete within the group's emission, so
        # consumers emitted at later steps see fully-written tiles.
        for t in range(ngroups + 3):
            if t < ngroups:
                for c in grp(t):
                    emit_load(c)
                cs = list(grp(t))
                for c in cs:
                    emit_layer(c, 0, c == cs[-1])
            if 1 <= t and t - 1 < ngroups:
                cs = list(grp(t - 1))
                for c in cs:
                    emit_layer(c, 1, c == cs[-1])
            if 2 <= t and t - 2 < ngroups:
                cs = list(grp(t - 2))
                for c in cs:
                    emit_layer(c, 2, c == cs[-1])
                for c in cs:
                    emit_h3_bf16(c)
            if 3 <= t and t - 3 < ngroups:
                for c in grp(t - 3):
                    emit_out(c)

        for p in (oppool, ppool, opool, scrpool, stkpool, bpool, hpool,
                  xpool, cpool):
            p.release()

    nc.compile()
    return nc


_PROGRAM_CACHE = {}


def _get_program(nsin, ngauss, nwrap, p_core=P_CORE, chunk=CHUNK):
    key = (tuple(nsin), tuple(ngauss), tuple(nwrap), p_core, chunk)
    if key not in _PROGRAM_CACHE:
        _PROGRAM_CACHE[key] = _build_program(nsin, ngauss, nwrap, p_core,
                                             chunk)
    return _PROGRAM_CACHE[key]


def make_in_maps(inputs, plan, p_core=P_CORE, n_cores=N_CORES):
    x = np.ascontiguousarray(np.asarray(inputs["inputs"], dtype=np.float32))
    pg = p_core // G
    cst = np.zeros((128, 448), dtype=np.float32)
    cst[0:G * N_IN, 0:128] = plan.lhsT[0]
    cst[:, 128:256] = plan.lhsT[1]
    cst[:, 256:384] = plan.lhsT[2]
    cst[:, 384:416] = plan.lhsT_out
    cst[:, 416:424] = plan.colblk
    in_maps = []
    for core in range(n_cores):
        xc = x[core * p_core:(core + 1) * p_core]
        xg = xc.reshape(G, pg, N_IN)
        xT = np.ascontiguousarray(
            xg.transpose(0, 2, 1).reshape(G * N_IN, pg))
        in_maps.append({"xT": xT, "cst": cst})
    return in_maps


def assemble_output(results, p_core=P_CORE, n_cores=N_CORES):
    pg = p_core // G
    out = np.empty((p_core * n_cores, N_OUT), dtype=np.float32)
    for core in range(n_cores):
        yT = results[core]["yT"]
        yc = yT.reshape(G, N_OUT, pg).transpose(0, 2, 1)
        out[core * p_core:(core + 1) * p_core] = yc.reshape(p_core, N_OUT)
    return out


def make_plan(inputs):
    return _Plan(
        inputs["inputs"],
        inputs["bias_in"], inputs["W1"], inputs["b1"], inputs["act1"],
        inputs["W2"], inputs["b2"], inputs["act2"],
        inputs["W3"], inputs["b3"], inputs["act3"],
        inputs["Wout"], inputs["bout"])


def run(inputs, trace=False, **spmd_kwargs):
    plan = make_plan(inputs)
    nc = _get_program(plan.nsin, plan.ngauss, plan.nwrap)
    in_maps = make_in_maps(inputs, plan)
    res = run_bass_kernel_spmd(nc, in_maps, list(range(N_CORES)),
                               trace=trace, **spmd_kwargs)
    return assemble_output(res.results), res


def kernel(**inputs) -> np.ndarray:
    out, _ = run(inputs, trace=False)
    return out


# revision 13
# speedup vs baseline: 1.5443x; 1.0923x over previous
"""CPPN MLP (12 -> 32 -> 32 -> 32 -> 3, per-node activations) on 8 TRN2 cores.

Data-parallel over the pixel axis; feature-major layout with G=4 pixel groups
on SBUF partitions (rhs partition 12*g+i holds feature i of group g for L1).

Per layer the 128 hidden rows (4 groups x 32 nodes, slot j -> partition
4*j+g) are class-sorted [gauss | tanh-class | sin].  Per chunk-layer the
ScalarE runs ONE dense main pass:
  Tanh over [0 : 128-4*nsin]  (per-row scale/bias operand columns: tanh
  (1,b), sigmoid (.5,.5b) [fold 0.5,0.5], identity (eps, eps*b) [fold 1/eps];
  junk on the gauss rows)
The sparse gauss/sin rows are DMA-stacked across k consecutive chunks into
dense SBUF stack tiles (k = 2 or 4 chosen per class so k*rows <= 128), where
per k chunks ONE dense pass each runs:
  sin:   ADD_RANGE_WRAP (DVE, custom ops cannot read PSUM - hence the DMA
         staging) then Sin(w + b) [bias column, stacked pattern]
  gauss: Square in-place (y=((z+b)/2)^2), Tanh, den=t+1 (DVE TS),
         r=recip_approx_fast(den) -> f32r, with exp(-u)=2/(1+tanh(u/2))-1
         folded (2,-1) into the next layer's weights
then k DMAs scatter the results back to each chunk's f32r h tile.

Matmuls are fp32r (full-rate fp32, ~1e-4) for the three hidden layers; the
output layer is bf16 (h3 copied to bf16 on DVE) quadrant-packed.
"""

import os
import sys

import numpy as np

_REPO = "/root/.axon_site/_ro/trn_rl_repo"
if _REPO not in sys.path and not os.path.isdir("/opt/trn_rl_repo"):
    sys.path.insert(0, _REPO)

import concourse.bacc as bacc
import concourse.bass as bass  # noqa: F401
import concourse.tile as tile
from concourse import mybir
from concourse.bass_utils import run_bass_kernel_spmd
from concourse.dve_ops import RECIP_APPROX_FAST_CONSTS as _RC
from concourse.dve_ops import RECIPROCAL_APPROX_FAST as _RF

# Pin the activation-function table to the single set containing every
# function this kernel uses ({Tanh, Square, Sin}).
_orig_get_tables = bacc.get_activation_tables


def _pinned_tables(arch):
    t = _orig_get_tables(arch)
    if "silu_and_others" in t:
        return {name: (funcs if name == "silu_and_others" else set())
                for name, funcs in t.items()}
    return t


bacc.get_activation_tables = _pinned_tables

F32 = mybir.dt.float32
F32R = mybir.dt.float32r
BF16 = mybir.dt.bfloat16
AF = mybir.ActivationFunctionType

P_TOTAL = 1024 * 1024
N_IN, H, N_OUT = 12, 32, 3
N_CORES = 8
P_CORE = P_TOTAL // N_CORES  # 131072
G = 4                        # pixel groups packed on partitions
PG = P_CORE // G             # 32768 pixels per group per core
CHUNK = 1024                 # pixels per group per chunk (2 PSUM banks)
MM_N = 512                   # matmul moving free dim (one PSUM bank)
GROUP = 4                    # chunks per pipeline group (stacking window)
ID_EPS = np.float32(2.0 ** -18)     # identity-via-tanh input scale
PI = float(np.pi)

# class codes determine the sort: 0 = gauss, 1 = tanh-class, 2 = sin
_CLS = {4: 0, 1: 1, 2: 1, 0: 1, 3: 2}


def _stack_k(rows, group):
    if rows == 0:
        return 0
    k = min(group, 128 // rows)
    return 4 if k >= 4 else (2 if k >= 2 else 1)


class _Plan:
    """Host-side folded weights + per-layer layouts. All float64 math."""

    def __init__(self, x, bias_in, W1, b1, act1, W2, b2, act2, W3, b3, act3,
                 Wout, bout):
        layers = [(np.asarray(W1, np.float64), np.asarray(b1, np.float64),
                   np.asarray(act1)),
                  (np.asarray(W2, np.float64), np.asarray(b2, np.float64),
                   np.asarray(act2)),
                  (np.asarray(W3, np.float64), np.asarray(b3, np.float64),
                   np.asarray(act3))]
        self.nsin, self.ngauss, self.nwrap = [], [], []
        self.perms = []
        self.lhsT = []

        # rigorous |z_true + b| bounds for wrap counts
        x64 = np.asarray(x, np.float64)
        bin64 = np.asarray(bias_in, np.float64)
        h0_max = np.abs(x64).max(axis=0) + np.abs(bin64)
        bounds = []
        maxh = h0_max
        for li, (W, b, act) in enumerate(layers):
            zb = np.abs(W).T @ maxh + np.abs(b)
            if li == 0:
                need = [n for n in range(H) if act[n] in (0, 3)]
                if need:
                    z1 = (x64 + bin64) @ W[:, need] + b[need]
                    zb[need] = np.abs(z1).max(axis=0)
            bounds.append(zb)
            mh = np.ones(H)
            for n in range(H):
                if act[n] == 0:
                    mh[n] = zb[n]
            maxh = mh

        in_alpha = np.ones(N_IN, dtype=np.float64)
        in_beta = bin64.copy()
        in_dim = N_IN
        in_layout = None
        colblk = np.zeros((128, 8), dtype=np.float64)

        for li, (W, b, act) in enumerate(layers):
            cls = np.array([_CLS[int(a)] for a in act])
            perm = np.argsort(cls, kind="stable")
            ns = int((cls == 2).sum())
            ng = int((cls == 0).sum())
            self.perms.append(perm)
            self.nsin.append(ns)
            self.ngauss.append(ng)

            W_eff = W * in_alpha[:, None]
            b_eff = b + in_beta @ W

            sin_nodes = [n for n in range(H) if act[n] == 3]
            if sin_nodes:
                m = max(bounds[li][n] + abs(b_eff[n]) for n in sin_nodes)
                self.nwrap.append(max(1, int(np.ceil((m / PI - 1.0) / 2.0))))
            else:
                self.nwrap.append(0)

            K = G * in_dim
            lt = np.zeros((K, 128), dtype=np.float64)
            for g in range(G):
                for j in range(H):
                    node = perm[j]
                    m_ = 4 * j + g
                    if li == 0:
                        rows = np.arange(in_dim) + in_dim * g
                        lt[rows, m_] = W_eff[:, node]
                    else:
                        for k_in in range(in_dim):
                            lt[in_layout[g][k_in], m_] = W_eff[k_in, node]
            self.lhsT.append(lt.astype(np.float32))

            # main-pass operand columns [tanh_scale, tanh_bias]: sin and
            # gauss rows stage eps*(z+b) through tanh (inverted on the stack)
            out_alpha = np.ones(H, dtype=np.float64)
            out_beta = np.zeros(H, dtype=np.float64)
            cb = 2 * li
            for j in range(H):
                node = perm[j]
                a = int(act[node])
                be = b_eff[node]
                for g in range(G):
                    m_ = 4 * j + g
                    if a == 1:
                        colblk[m_, cb + 0] = 1.0
                        colblk[m_, cb + 1] = be
                    elif a == 2:
                        colblk[m_, cb + 0] = 0.5
                        colblk[m_, cb + 1] = 0.5 * be
                    else:   # identity, gauss, sin: eps*(z+b)
                        colblk[m_, cb + 0] = float(ID_EPS)
                        colblk[m_, cb + 1] = float(ID_EPS) * be
                if a == 4:
                    out_alpha[node], out_beta[node] = 2.0, -1.0
                elif a == 2:
                    out_alpha[node], out_beta[node] = 0.5, 0.5
                elif a == 0:
                    out_alpha[node], out_beta[node] = 1.0 / float(ID_EPS), 0.0

            in_dim = H
            in_layout = [[4 * j + g for j in range(H)] for g in range(G)]
            in_alpha = out_alpha[perm]
            in_beta = out_beta[perm]
            if li < 2:
                layers[li + 1] = (np.asarray(layers[li + 1][0])[perm, :],
                                  layers[li + 1][1], layers[li + 1][2])
            else:
                self._wout_perm = perm

        # output layer (quadrant-packed, bf16)
        Wo = np.asarray(Wout, dtype=np.float64)[self._wout_perm, :]
        bo = np.asarray(bout, dtype=np.float64)
        Wo_eff = Wo * in_alpha[:, None]
        bo_eff = bo + in_beta @ Wo
        lt = np.zeros((128, 32), dtype=np.float64)
        for g in range(G):
            for j in range(H):
                kpart = in_layout[g][j]
                for o in range(N_OUT):
                    lt[kpart, 3 * g + o] = Wo_eff[j, o]
        self.lhsT_out = lt.astype(np.float32)
        out_bias = np.zeros(128, dtype=np.float64)
        for q in range(4):
            for g in range(G):
                for o in range(N_OUT):
                    out_bias[32 * q + 3 * g + o] = bo_eff[o]
        colblk[:, 6] = out_bias
        self.colblk = colblk.astype(np.float32)


def _build_program(nsin, ngauss, nwrap, p_core=P_CORE, chunk=CHUNK):
    """Program structure depends only on (nsin, ngauss, nwrap) per layer."""
    pg = p_core // G
    nchunk = pg // chunk
    nhalf = chunk // MM_N
    group = min(GROUP, nchunk)
    ngroups = (nchunk + group - 1) // group
    assert chunk % MM_N == 0 and pg % chunk == 0

    nc = bacc.Bacc("TRN2", target_bir_lowering=False, debug=False,
                   num_devices=N_CORES)
    xT = nc.dram_tensor("xT", [G * N_IN, pg], F32, kind="ExternalInput").ap()
    cst = nc.dram_tensor("cst", [128, 448], F32, kind="ExternalInput").ap()
    yT = nc.dram_tensor("yT", [12, pg], F32, kind="ExternalOutput").ap()

    with tile.TileContext(nc) as tc:
        cpool = tc.alloc_tile_pool(name="consts", bufs=1)
        wst_t = cpool.tile([128, 416], F32R, tag="wst")
        wob_t = cpool.tile([128, 32], BF16, tag="wob")
        cc_t = cpool.tile([128, 8], F32, tag="cc")
        nc.gpsimd.dma_start(out=wst_t[:], in_=cst[:, 0:416])
        nc.sync.dma_start(out=cc_t[:], in_=cst[:, 416:424])
        # bf16 copy of the output stationary (DVE converts)
        nc.vector.tensor_copy(wob_t[:], wst_t[:, 384:416].bitcast(F32))
        w_tiles = [wst_t[:, 0:128], wst_t[:, 128:256], wst_t[:, 256:384]]
        col_t = cc_t[:, 0:8]

        xpool = tc.alloc_tile_pool(name="xin", bufs=2 * group)
        hpool = tc.alloc_tile_pool(name="h", bufs=3 * group + 2)
        bpool = tc.alloc_tile_pool(name="hb16", bufs=group + 2)
        stkpool = tc.alloc_tile_pool(name="stk", bufs=4)
        scrpool = tc.alloc_tile_pool(name="scr", bufs=2)
        opool = tc.alloc_tile_pool(name="osb", bufs=2)
        ppool = tc.alloc_tile_pool(name="psum", bufs=3, space="PSUM")
        oppool = tc.alloc_tile_pool(name="psum_o", bufs=2, space="PSUM")

        h_live = {}
        pso_live = {}
        sin_pend = {0: [], 1: [], 2: []}    # li -> [(slot_b, h_tile)]
        sin_stk = {}
        gau_pend = {0: [], 1: [], 2: []}
        gau_stk = {}

        def emit_load(c):
            x_t = xpool.tile([G * N_IN, chunk], F32R, tag="x")
            nc.gpsimd.dma_start(out=x_t[:],
                                in_=xT[:, c * chunk:(c + 1) * chunk])
            h_live[(c, 0)] = x_t

        def flush_sin(li):
            pend = sin_pend[li]
            if not pend:
                return
            S4 = 4 * nsin[li]
            used = len(pend) * S4
            stk = sin_stk[li]
            # stack holds eps*(z+b); unscale, wrap into [-pi, pi], Sin
            wscr = scrpool.tile([128, chunk], F32, tag="wscr")
            nc.vector.tensor_scalar(
                wscr[0:used, :], stk[0:used, :], 1.0 / float(ID_EPS), None,
                mybir.AluOpType.mult)
            src, dst = wscr, stk
            for _ in range(nwrap[li]):
                nc.vector.add_range_wrap(dst[0:used, :], src[0:used, :],
                                         0.0, PI, 2 * PI)
                src, dst = dst, src
            nc.scalar.activation(
                dst[0:used, :].bitcast(F32R), src[0:used, :], AF.Sin)
            for b, h in pend:
                nc.scalar.dma_start(
                    out=h[128 - S4:128, :],
                    in_=dst[b * S4:(b + 1) * S4, :].bitcast(F32R))
            pend.clear()

        def flush_gauss(li):
            pend = gau_pend[li]
            if not pend:
                return
            G4 = 4 * ngauss[li]
            used = len(pend) * G4
            stk = gau_stk[li]
            # stack holds eps*(z+b): y = ((z+b)/2)^2 in place, t = tanh(y)
            nc.scalar.activation(
                stk[0:used, :], stk[0:used, :], AF.Square,
                scale=0.5 / float(ID_EPS))
            nc.scalar.activation(
                stk[0:used, :], stk[0:used, :], AF.Tanh)
            den = scrpool.tile([128, chunk], F32, tag="den")
            nc.vector.tensor_scalar(
                den[0:used, :], stk[0:used, :], 1.0, None,
                mybir.AluOpType.add)
            nc.vector._custom_dve(
                _RF, out=stk[0:used, :].bitcast(F32R), in0=den[0:used, :],
                s0=_RC["s0"], s1=_RC["s1"], imm2=_RC["imm2"])
            for b, h in pend:
                nc.sync.dma_start(
                    out=h[0:G4, :],
                    in_=stk[b * G4:(b + 1) * G4, :].bitcast(F32R))
            pend.clear()

        def emit_layer(c, li, last_in_group):
            h_prev = h_live.pop((c, li))
            ns, ng = nsin[li], ngauss[li]
            S4, G4 = 4 * ns, 4 * ng
            ks = _stack_k(S4, group)
            kg = _stack_k(G4, group)
            kdim = G * N_IN if li == 0 else 128
            cb = 2 * li

            ps = ppool.tile([128, chunk], F32, tag="pre")
            wt = w_tiles[li]
            for hh in range(nhalf):
                sl = slice(hh * MM_N, (hh + 1) * MM_N)
                nc.tensor.matmul(
                    ps[:, sl],
                    wt[0:kdim, :],
                    h_prev[0:kdim, sl],
                    start=True, stop=True,
                )
            h = hpool.tile([128, chunk], F32R, tag="h")
            # one dense main pass over all rows: tanh-class real values;
            # gauss/sin rows staged as tanh(eps*(z+b)) ~= eps*(z+b)
            nc.scalar.activation(
                h[:, :], ps[:, :], AF.Tanh,
                bias=col_t[:, cb + 1:cb + 2],
                scale=col_t[:, cb + 0:cb + 1],
            )
            # stack the sparse class rows (SBUF -> SBUF)
            if S4:
                if not sin_pend[li]:
                    sin_stk[li] = stkpool.tile([128, chunk], F32, tag="sstk",
                                               name="sstk")
                b = len(sin_pend[li])
                nc.scalar.dma_start(out=sin_stk[li][b * S4:(b + 1) * S4, :],
                                    in_=h[128 - S4:128, :].bitcast(F32))
                sin_pend[li].append((b, h))
                if len(sin_pend[li]) == ks or last_in_group:
                    flush_sin(li)
            if G4:
                if not gau_pend[li]:
                    gau_stk[li] = stkpool.tile([128, chunk], F32, tag="gstk",
                                               name="gstk")
                b = len(gau_pend[li])
                nc.sync.dma_start(out=gau_stk[li][b * G4:(b + 1) * G4, :],
                                  in_=h[0:G4, :].bitcast(F32))
                gau_pend[li].append((b, h))
                if len(gau_pend[li]) == kg or last_in_group:
                    flush_gauss(li)
            h_live[(c, li + 1)] = h

        def emit_h3_bf16(c):
            h3 = h_live.pop((c, 3))
            hb = bpool.tile([128, chunk], BF16, tag="hb")
            nc.vector.tensor_copy(hb[:], h3[:].bitcast(F32))
            h_live[(c, 4)] = hb

        def emit_out(c):
            h_prev = h_live.pop((c, 4))
            q0 = 2 * (c % 2)
            if q0 == 0:
                pso_live[c // 2] = oppool.tile([128, MM_N], F32, tag="preo",
                                               name="pso")
            pso = pso_live[c // 2]
            for hh in range(nhalf):
                q = q0 + hh
                nc.tensor.matmul(
                    pso[32 * q:32 * q + 32, :],
                    wob_t,
                    h_prev[:, hh * MM_N:(hh + 1) * MM_N],
                    start=True, stop=True,
                    tile_position=(0, 32 * q),
                )
            if q0 == 2:
                pso_live.pop(c // 2)
                osb = opool.tile([128, MM_N], F32, tag="osb")
                nc.scalar.activation(
                    osb[:], pso[:], AF.Tanh,
                    bias=col_t[:, 6:7],
                )
                base = (c - 1) * chunk
                for q in range(4):
                    nc.sync.dma_start(
                        out=yT[:, base + q * MM_N: base + (q + 1) * MM_N],
                        in_=osb[32 * q:32 * q + 12, :])

        def grp(t):
            return range(t * group, min((t + 1) * group, nchunk))

        # group-granular software pipeline: step t emits loads+L1 for group
        # t, L2 for t-1, L3 for t-2, out for t-3. All stacked-class writes
        # to a group's h tiles complete within the group's emission, so
        # consumers emitted at later steps see fully-written tiles.
        for t in range(ngroups + 3):
            if t < ngroups:
                for c in grp(t):
                    emit_load(c)
                cs = list(grp(t))
                for c in cs:
                    emit_layer(c, 0, c == cs[-1])
            if 1 <= t and t - 1 < ngroups:
                cs = list(grp(t - 1))
                for c in cs:
                    emit_layer(c, 1, c == cs[-1])
            if 2 <= t and t - 2 < ngroups:
                cs = list(grp(t - 2))
                for c in cs:
                    emit_layer(c, 2, c == cs[-1])
                for c in cs:
                    emit_h3_bf16(c)
            if 3 <= t and t - 3 < ngroups:
                for c in grp(t - 3):
                    emit_out(c)

        for p in (oppool, ppool, opool, scrpool, stkpool, bpool, hpool,
                  xpool, cpool):
            p.release()

    nc.compile()
    return nc


_PROGRAM_CACHE = {}


def _get_program(nsin, ngauss, nwrap, p_core=P_CORE, chunk=CHUNK):
    key = (tuple(nsin), tuple(ngauss), tuple(nwrap), p_core, chunk)
    if key not in _PROGRAM_CACHE:
        _PROGRAM_CACHE[key] = _build_program(nsin, ngauss, nwrap, p_core,
                                             chunk)
    return _PROGRAM_CACHE[key]


def make_in_maps(inputs, plan, p_core=P_CORE, n_cores=N_CORES):
    x = np.ascontiguousarray(np.asarray(inputs["inputs"], dtype=np.float32))
    pg = p_core // G
    cst = np.zeros((128, 448), dtype=np.float32)
    cst[0:G * N_IN, 0:128] = plan.lhsT[0]
    cst[:, 128:256] = plan.lhsT[1]
    cst[:, 256:384] = plan.lhsT[2]
    cst[:, 384:416] = plan.lhsT_out
    cst[:, 416:424] = plan.colblk
    in_maps = []
    for core in range(n_cores):
        xc = x[core * p_core:(core + 1) * p_core]
        xg = xc.reshape(G, pg, N_IN)
        xT = np.ascontiguousarray(
            xg.transpose(0, 2, 1).reshape(G * N_IN, pg))
        in_maps.append({"xT": xT, "cst": cst})
    return in_maps


def assemble_output(results, p_core=P_CORE, n_cores=N_CORES):
    pg = p_core // G
    out = np.empty((p_core * n_cores, N_OUT), dtype=np.float32)
    for core in range(n_cores):
        yT = results[core]["yT"]
        yc = yT.reshape(G, N_OUT, pg).transpose(0, 2, 1)
        out[core * p_core:(core + 1) * p_core] = yc.reshape(p_core, N_OUT)
    return out


def make_plan(inputs):
    return _Plan(
        inputs["inputs"],
        inputs["bias_in"], inputs["W1"], inputs["b1"], inputs["act1"],
        inputs["W2"], inputs["b2"], inputs["act2"],
        inputs["W3"], inputs["b3"], inputs["act3"],
        inputs["Wout"], inputs["bout"])


def run(inputs, trace=False, **spmd_kwargs):
    plan = make_plan(inputs)
    nc = _get_program(plan.nsin, plan.ngauss, plan.nwrap)
    in_maps = make_in_maps(inputs, plan)
    res = run_bass_kernel_spmd(nc, in_maps, list(range(N_CORES)),
                               trace=trace, **spmd_kwargs)
    return assemble_output(res.results), res


def kernel(**inputs) -> np.ndarray:
    out, _ = run(inputs, trace=False)
    return out
